# revision 19
# baseline (speedup 1.0000x reference)
"""Self-contained TRN2 Bass kernel for the 2-layer multi-head GAT problem.

kernel(**inputs) -> np.ndarray [100000, 40] float32 (log_softmax outputs).

Strategy: dst-sharded graph parallelism across 8 NeuronCores. Dense phases
compute packed bf16 node tables that are AllGathered; edge phases gather
source rows with dma_gather (512B rows: 240 bf16 h + 16 fp8 h + 8 bf16 a_s,
channel-major so the per-head ee broadcast hits the DVE 2x packed mode),
aggregate per 128-dst tile via one-hot matmuls in PSUM (one-hot built
chunk-dim-last for 2x is_equal), segment softmax folded into a final
normalize (exp-shift M is softmax-invariant). Gather calls are merged two
dst-tiles at a time to halve GPSIMD descriptor-prep fixed costs; log_softmax
Ln is batched across tiles to avoid activation-table thrash.
"""

import math
import sys
from contextlib import ExitStack
from dataclasses import dataclass, field

import numpy as np

sys.path.insert(0, "/opt/trn_rl_repo")

import concourse.bacc as bacc
import concourse.bass as bass
import concourse.tile as tile
from concourse import mybir
from concourse.masks import make_identity

F32 = mybir.dt.float32
BF16 = mybir.dt.bfloat16
FP8 = mybir.dt.float8e4
I16 = mybir.dt.int16

NBF = 240          # h channels stored in bf16 (channel-major, heads innermost)
NF8 = 16           # h channels stored in fp8 (channels 30,31 x 8 heads)


@dataclass
class Cfg:
    n_nodes: int = 100000
    f_in: int = 128
    hid: int = 32
    heads: int = 8
    classes: int = 40
    n_cores: int = 8
    tiles_per_core: int = 100
    n_blocks: int = 4
    m1: float = 16.0       # exp-shift layer 1
    m2: float = 16.0       # exp-shift layer 2
    neg_slope: float = 0.2
    repeat: int = 1
    sim1: bool = False     # build single-core program (collectives -> DMA)
    # host-derived static bookkeeping (set by prep_host_data)
    c_tb: object = None            # [T, B] chunks per (tile, block)
    pairs: object = None           # list of per-pair dicts
    total_chunks: int = 0
    total_idx16: int = 0
    cb_pair_max: int = 0
    run_max: int = 0

    @property
    def d1(self):
        return self.heads * self.hid

    @property
    def n_pad(self):
        return self.n_cores * self.tiles_per_core * 128

    @property
    def nodes_per_core(self):
        return self.tiles_per_core * 128

    @property
    def block_rows(self):
        assert self.n_pad % self.n_blocks == 0
        return self.n_pad // self.n_blocks

    @property
    def row1(self):
        return 256  # bf16 slots: 240 h-bf16 | 8 a_s | 8 slots (=16 fp8 h)

    @property
    def row2(self):
        return 128  # bf16 slots: 32 h2 | 1 a_s2 | pad

    @property
    def n_pairs(self):
        return self.tiles_per_core // 2


def degree_balance_perm(dst: np.ndarray, cfg: Cfg) -> np.ndarray:
    """pi[old_id] = new_id; in-degrees balanced across 128-node tiles via
    snake round-robin over tiles in descending-degree order. Vectorized."""
    n, npad = cfg.n_nodes, cfg.n_pad
    deg = np.bincount(dst, minlength=n).astype(np.int64) + 1
    order = np.argsort(-deg, kind="stable")
    n_tiles = npad // 128
    i = np.arange(n, dtype=np.int64)
    rnd = i // n_tiles
    pos = i % n_tiles
    tl = np.where(rnd % 2 == 0, pos, n_tiles - 1 - pos)
    slot = rnd
    assert slot.max() < 128
    pi = np.empty(npad, dtype=np.int64)
    pi[order] = tl * 128 + slot
    used = np.zeros(npad, dtype=bool)
    used[pi[:n]] = True
    pi[n:] = np.flatnonzero(~used)
    return pi


@dataclass
class HostData:
    perm: np.ndarray
    inv_perm: np.ndarray
    per_core: list


def wrap16(idx_1d: np.ndarray) -> np.ndarray:
    """[n] -> [16, n/16] wrapped (j at [j%16, j//16]), tiled to [128, n/16]."""
    n = idx_1d.shape[0]
    assert n % 16 == 0
    w = idx_1d.reshape(n // 16, 16).T.copy()
    return np.tile(w, (8, 1))


def h_col_perm(cfg: Cfg) -> np.ndarray:
    """perm[new_col] = orig_col mapping 256 h cols to channel-major layout:
    [c*8+h for c<30] + [16 fp8: c in 30,31]. orig col = h*32 + c."""
    H, C = cfg.heads, cfg.hid
    cols = []
    for c in range(30):
        for h in range(H):
            cols.append(h * C + c)
    for c in (30, 31):
        for h in range(H):
            cols.append(h * C + c)
    return np.array(cols, dtype=np.int64)


def prep_host_data(cfg: Cfg, inputs: dict) -> HostData:
    import ml_dtypes
    n, npad = cfg.n_nodes, cfg.n_pad
    T, B, H, C = cfg.tiles_per_core, cfg.n_blocks, cfg.heads, cfg.hid
    x = np.asarray(inputs["x"], dtype=np.float32)
    ei = np.asarray(inputs["edge_index"])
    src0 = ei[0].astype(np.int64)
    dst0 = ei[1].astype(np.int64)
    loops = np.arange(n, dtype=np.int64)
    src0 = np.concatenate([src0, loops])
    dst0 = np.concatenate([dst0, loops])

    perm = degree_balance_perm(dst0, cfg)
    inv_perm = np.argsort(perm)
    src = perm[src0]
    dst = perm[dst0]

    # --- weights ---
    W1 = np.asarray(inputs["W1"], dtype=np.float64)
    att_s1 = np.asarray(inputs["att_s1"], dtype=np.float64)
    att_d1 = np.asarray(inputs["att_d1"], dtype=np.float64)
    b1 = np.asarray(inputs["b1"], dtype=np.float32)
    W2 = np.asarray(inputs["W2"], dtype=np.float64)
    att_s2 = np.asarray(inputs["att_s2"], dtype=np.float64)
    att_d2 = np.asarray(inputs["att_d2"], dtype=np.float64)
    b2 = np.asarray(inputs["b2"], dtype=np.float32)
    Wout = np.asarray(inputs["Wout"], dtype=np.float32)
    bout = np.asarray(inputs["bout"], dtype=np.float32)

    Ws1 = np.zeros((cfg.f_in, H), dtype=np.float64)
    Wd1 = np.zeros((cfg.f_in, H), dtype=np.float64)
    for h in range(H):
        Ws1[:, h] = W1[:, h * C:(h + 1) * C] @ att_s1[h]
        Wd1[:, h] = W1[:, h * C:(h + 1) * C] @ att_d1[h]
    hperm = h_col_perm(cfg)
    W1p = W1[:, hperm]                      # [F, 256] channel-major
    # W1ext cols: 0:240 h-bf16 | 240:248 a_s | 248:264 h-fp8 | 264:272 a_d
    W1ext = np.concatenate(
        [W1p[:, :NBF], Ws1, W1p[:, NBF:], Wd1], axis=1).astype(np.float32)
    # W2ext rows permuted to match elu output order [240 | 16]
    W2p = W2[hperm, :]                      # [256, C]
    s2 = W2p @ att_s2[0]
    d2 = W2p @ att_d2[0]
    W2ext = np.concatenate([W2p, s2[:, None], d2[:, None]],
                           axis=1).astype(np.float32)   # [256, C+2]
    b1p = b1[hperm]

    # --- per-core edge prep ---
    npc = cfg.nodes_per_core
    core_of_edge = dst // npc
    per_core_raw = []
    for k in range(cfg.n_cores):
        m = core_of_edge == k
        es, ed = src[m], dst[m]
        tile_of = (ed - k * npc) // 128
        block_of = es // cfg.block_rows
        key = tile_of * B + block_of
        order = np.argsort(key, kind="stable")
        es, ed, key = es[order], ed[order], key[order]
        counts = np.bincount(key, minlength=T * B)
        per_core_raw.append(dict(es=es, ed=ed, counts=counts, k=k))

    all_counts = np.stack([pc["counts"] for pc in per_core_raw])  # [cores,T*B]
    c_tb = ((all_counts.max(axis=0) + 127) // 128).astype(np.int64)
    c_tb = np.maximum(c_tb, 1).reshape(T, B)
    cfg.c_tb = c_tb
    cfg.run_max = int(c_tb.max())

    # --- pair bookkeeping ---
    NP = cfg.n_pairs
    pairs = []
    choff = 0
    i16off = 0
    for P in range(NP):
        t0, t1 = 2 * P, 2 * P + 1
        runs = []          # (b, t, run_chunks, chunk_off_in_pair)
        off = 0
        for b in range(B):
            runs.append((b, t0, int(c_tb[t0, b]), off))
            off += int(c_tb[t0, b])
            runs.append((b, t1, int(c_tb[t1, b]), off))
            off += int(c_tb[t1, b])
        cbP = off
        # one gather call per (tile, block): SWDGE ring caps a call at 1024
        # descriptors, so pair-merged calls (~1536 idx) are not possible.
        calls = []
        for b, t, rl, roff in runs:
            calls.append(dict(b=b, t=t, r=rl, roff=roff, i16off=i16off))
            i16off += rl * 8
        tile_chunks = {t0: [], t1: []}
        for b, t, rl, roff in runs:
            tile_chunks[t].extend(range(roff, roff + rl))
        chunk_tile = np.empty(cbP, dtype=np.int64)
        for b, t, rl, roff in runs:
            chunk_tile[roff:roff + rl] = t
        pairs.append(dict(P=P, choff=choff, cbP=cbP, runs=runs, calls=calls,
                          tile_chunks=tile_chunks, chunk_tile=chunk_tile))
        choff += cbP
    cfg.pairs = pairs
    cfg.total_chunks = choff
    cfg.total_idx16 = i16off
    cfg.cb_pair_max = max(p["cbP"] for p in pairs)

    # --- per-core streams: idx (wrap16), dstloc (bf16), ncnt ---
    per_core_arrays = []
    for pc in per_core_raw:
        es, ed, counts, k = pc["es"], pc["ed"], pc["counts"], pc["k"]
        starts = np.concatenate([[0], np.cumsum(counts)])
        idx_wr = np.zeros((128, cfg.total_idx16), dtype=np.int16)
        dl = np.full(cfg.total_chunks * 128, -1.0, dtype=np.float32)
        ncnt = np.zeros(cfg.n_pairs * 2 * cfg.n_blocks, dtype=np.int32)
        for p in pairs:
            P = p["P"]
            for ci, call in enumerate(p["calls"]):
                b, t, rl = call["b"], call["t"], call["r"]
                gidx = t * B + b
                cnt = int(starts[gidx + 1] - starts[gidx])
                st = np.full(rl * 128, -1, dtype=np.int16)
                st[:cnt] = (es[starts[gidx]:starts[gidx + 1]]
                            - b * cfg.block_rows).astype(np.int16)
                ncnt[P * 2 * B + ci] = cnt
                idx_wr[:, call["i16off"]:call["i16off"] + rl * 8] = wrap16(st)
            # dstloc in pair-chunk order
            for b, t, rl, roff in p["runs"]:
                g = t * B + b
                cnt = int(starts[g + 1] - starts[g])
                locd = (ed[starts[g]:starts[g + 1]]
                        - k * npc - t * 128).astype(np.float32)
                base = (p["choff"] + roff) * 128
                dl[base:base + cnt] = locd
        dl2 = dl.reshape(cfg.total_chunks, 128)
        dlp = np.ascontiguousarray(dl2.T).astype(ml_dtypes.bfloat16)
        per_core_arrays.append(dict(idx=idx_wr, dlp=dlp,
                                    ncnt=ncnt.reshape(1, -1)))

    # --- dense inputs per core ---
    x_pad = np.zeros((npad, cfg.f_in), dtype=np.float32)
    x_pad[perm[:n]] = x
    xT = x_pad.T.astype(ml_dtypes.bfloat16)

    iota_row = np.arange(128, dtype=np.float32)[None, :]
    RL = cfg.run_max
    iota_rep = np.repeat(np.arange(128, dtype=np.float32), RL)[None, :]

    ck = min(128, cfg.d1)
    nk = cfg.d1 // ck
    W2chunk = np.ascontiguousarray(
        W2ext.reshape(nk, ck, C + 2).transpose(1, 0, 2)).reshape(ck, -1)
    for k, arrs in enumerate(per_core_arrays):
        arrs["xT"] = np.ascontiguousarray(xT[:, k * npc:(k + 1) * npc])
        arrs["W1ext"] = W1ext.astype(ml_dtypes.bfloat16)
        arrs["W2ext"] = W2chunk.astype(ml_dtypes.bfloat16)
        arrs["Wout"] = Wout.astype(ml_dtypes.bfloat16)
        arrs["b1"] = b1p[None, :].astype(np.float32)
        arrs["b2"] = b2[None, :].astype(np.float32)
        arrs["bout"] = bout[None, :].astype(np.float32)
        arrs["iota_row"] = iota_row
        arrs["iota_rep"] = iota_rep

    return HostData(perm=perm, inv_perm=inv_perm, per_core=per_core_arrays)


# ============================== device program ==============================

def build_program(cfg: Cfg, debug: bool = False):
    ndev = 1 if cfg.sim1 else cfg.n_cores
    nc = bacc.Bacc("TRN2", target_bir_lowering=False, debug=debug,
                   num_devices=ndev, num_swdge_queues=4)
    T, B, H, C = cfg.tiles_per_core, cfg.n_blocks, cfg.heads, cfg.hid
    D1 = cfg.d1
    npc, npad = cfg.nodes_per_core, cfg.n_pad
    row1, row2 = cfg.row1, cfg.row2
    F = cfg.f_in
    NP = cfg.n_pairs
    CBP = cfg.cb_pair_max
    RL = cfg.run_max
    groups = [list(range(cfg.n_cores))]

    # ---- inputs ----
    xT = nc.dram_tensor("xT", [F, npc], BF16, kind="ExternalInput")
    W1ext = nc.dram_tensor("W1ext", [F, D1 + 2 * H], BF16, kind="ExternalInput")
    ck = min(128, D1)
    nk = D1 // ck
    W2ext = nc.dram_tensor("W2ext", [ck, nk * (C + 2)], BF16, kind="ExternalInput")
    Wout = nc.dram_tensor("Wout", [C, cfg.classes], BF16, kind="ExternalInput")
    b1 = nc.dram_tensor("b1", [1, D1], F32, kind="ExternalInput")
    b2 = nc.dram_tensor("b2", [1, C], F32, kind="ExternalInput")
    bout = nc.dram_tensor("bout", [1, cfg.classes], F32, kind="ExternalInput")
    iota_row = nc.dram_tensor("iota_row", [1, 128], F32, kind="ExternalInput")
    iota_rep = nc.dram_tensor("iota_rep", [1, 128 * RL], F32, kind="ExternalInput")
    idx = nc.dram_tensor("idx", [128, cfg.total_idx16], I16, kind="ExternalInput")
    ncnt = nc.dram_tensor("ncnt", [1, NP * 2 * B], mybir.dt.int32,
                          kind="ExternalInput")
    dlp = nc.dram_tensor("dlp", [128, cfg.total_chunks], BF16, kind="ExternalInput")

    # ---- internal / output ----
    tab1_sh = nc.dram_tensor("tab1_sh", [npc, row1], BF16)
    tab1 = nc.dram_tensor("tab1", [npad, row1], BF16, addr_space="Shared")
    tab2_sh = nc.dram_tensor("tab2_sh", [npc, row2], BF16)
    tab2 = nc.dram_tensor("tab2", [npad, row2], BF16, addr_space="Shared")
    out = nc.dram_tensor("out", [npc, cfg.classes], F32, kind="ExternalOutput")

    with tile.TileContext(nc, num_cores=ndev) as tc, ExitStack() as ctx:
        consts = ctx.enter_context(tc.tile_pool(name="consts", bufs=1))

        W1e_sb = consts.tile([F, D1 + 2 * H], BF16)
        nc.sync.dma_start(out=W1e_sb, in_=W1ext[:])
        W2e_sb = consts.tile([ck, nk, C + 2], BF16)
        nc.sync.dma_start(out=W2e_sb, in_=W2ext[:].rearrange("p (a c) -> p a c", a=nk))
        Wout_sb = consts.tile([C, cfg.classes], BF16)
        nc.sync.dma_start(out=Wout_sb, in_=Wout[:])
        b1_sb = consts.tile([128, D1], F32)
        nc.sync.dma_start(out=b1_sb, in_=bass.AP(
            tensor=b1.ap().tensor, offset=0, ap=[[0, 128], [1, D1]]))
        b2_sb = consts.tile([128, C], F32)
        nc.sync.dma_start(out=b2_sb, in_=bass.AP(
            tensor=b2.ap().tensor, offset=0, ap=[[0, 128], [1, C]]))
        bout_sb = consts.tile([128, cfg.classes], F32)
        nc.sync.dma_start(out=bout_sb, in_=bass.AP(
            tensor=bout.ap().tensor, offset=0, ap=[[0, 128], [1, cfg.classes]]))
        iota_sb = consts.tile([128, 128], F32)
        nc.sync.dma_start(out=iota_sb, in_=bass.AP(
            tensor=iota_row.ap().tensor, offset=0, ap=[[0, 128], [1, 128]]))
        iotar_f32 = consts.tile([128, 128 * RL], F32)
        nc.sync.dma_start(out=iotar_f32, in_=bass.AP(
            tensor=iota_rep.ap().tensor, offset=0, ap=[[0, 128], [1, 128 * RL]]))
        iotar_sb = consts.tile([128, 128, RL], BF16)
        nc.vector.tensor_copy(
            iotar_sb, iotar_f32[:].rearrange("p (d r) -> p d r", d=128))
        ident_sb = consts.tile([128, 128], BF16)
        make_identity(nc, ident_sb)
        zero_b = consts.tile([128, 1], F32)
        nc.vector.memset(zero_b, 0.0)
        mneg1_b = consts.tile([128, 1], F32)
        nc.vector.memset(mneg1_b, -cfg.m1)
        mneg2_b = consts.tile([128, 1], F32)
        nc.vector.memset(mneg2_b, -cfg.m2)
        idx_sb = consts.tile([128, cfg.total_idx16], I16)
        nc.sync.dma_start(out=idx_sb, in_=idx[:])
        ncnt_sb = consts.tile([1, NP * 2 * B], mybir.dt.int32)
        nc.sync.dma_start(out=ncnt_sb, in_=ncnt[:])
        gcnt_regs = [nc.gpsimd.alloc_register(f"gcnt{i}")
                     for i in range(3 * 2 * B)]
        dlp_sb = consts.tile([128, cfg.total_chunks], BF16)
        nc.sync.dma_start(out=dlp_sb, in_=dlp[:])
        a_d1_sb = consts.tile([128, T, 2, H], BF16)
        a_d2_sb = consts.tile([128, T, 2, 1], BF16)
        zs_sb = consts.tile([128, T, cfg.classes], BF16)
        ssum_sb = consts.tile([128, T], F32)

        for _rep in range(cfg.repeat):
            # ---------------- phase A: dense + table1 ----------------
            with tc.tile_pool(name="phA", bufs=3) as pa, \
                 tc.tile_pool(name="phA_x", bufs=1) as pax, \
                 tc.tile_pool(name="phA_ps", bufs=2, space="PSUM") as pap:
                xT_sb = pax.tile([F, npc], BF16)
                nc.sync.dma_start(out=xT_sb, in_=xT[:])
                for t in range(T):
                    ps = pap.tile([128, D1 + 2 * H], F32, tag="psA")
                    nc.tensor.matmul(out=ps, lhsT=xT_sb[:, t * 128:(t + 1) * 128],
                                     rhs=W1e_sb[:], start=True, stop=True)
                    row = pa.tile([128, row1], BF16, tag="rowA")
                    nc.scalar.copy(row[:, 0:NBF + H], ps[:, 0:NBF + H])
                    tmp8 = pa.tile([128, NF8], FP8, tag="tmp8")
                    nc.scalar.copy(tmp8, ps[:, NBF + H:NBF + H + NF8])
                    nc.vector.tensor_copy(row[:, NBF + H:row1],
                                          tmp8[:].bitcast(BF16))
                    nc.vector.tensor_copy(a_d1_sb[:, t, 0, :],
                                          ps[:, NBF + H + NF8:])
                    nc.vector.tensor_tensor(out=a_d1_sb[:, t, 1, :],
                                            in0=ps[:, NBF + H + NF8:],
                                            in1=a_d1_sb[:, t, 0, :],
                                            op=mybir.AluOpType.subtract)
                    nc.sync.dma_start(out=tab1_sh[t * 128:(t + 1) * 128, :],
                                      in_=row[:])

            if cfg.n_cores == 1:
                nc.sync.dma_start(out=tab1[:], in_=tab1_sh[:])
            elif cfg.sim1:
                for k in range(cfg.n_cores):
                    nc.sync.dma_start(out=tab1[k * npc:(k + 1) * npc, :],
                                      in_=tab1_sh[:])
            else:
                nc.gpsimd.collective_compute(
                    "AllGather", mybir.AluOpType.bypass, groups,
                    ins=[tab1_sh[:]], outs=[tab1[:]])

            # ---------------- edge phases ----------------
            def edge_phase(tab, rowN, a_d_sb, HN, mneg_bias, msg_w, as_off,
                           post_tile):
                """rowN: table row width; HN: heads; msg_w: msg cols (DN+HN);
                as_off: column of a_s within the gathered row."""
                DN = msg_w - HN
                with tc.tile_pool(name="phB_g", bufs=2) as pgath, \
                     tc.tile_pool(name="phB_s", bufs=2) as psalt, \
                     tc.tile_pool(name="phB_m", bufs=1) as pmsg, \
                     tc.tile_pool(name="phB_s2", bufs=2) as ps2p, \
                     tc.tile_pool(name="phB_e", bufs=2) as pep, \
                     tc.tile_pool(name="phB_post", bufs=2) as ppost, \
                     tc.tile_pool(name="phB_ps2", bufs=2, space="PSUM") as pps2, \
                     tc.tile_pool(name="phB_psE", bufs=1, space="PSUM") as ppsE, \
                     tc.tile_pool(name="phB_psG", bufs=2, space="PSUM") as ppsG, \
                     tc.tile_pool(name="phB_psX", bufs=1, space="PSUM") as ppsX:
                    for p in cfg.pairs:
                        P, cbP = p["P"], p["cbP"]
                        g = pgath.tile([128, CBP, rowN], BF16, tag="gath")
                        if P < 2:
                            nc.vector.memset(g, 0.0)
                        for ci, call in enumerate(p["calls"]):
                            b, rl, roff = call["b"], call["r"], call["roff"]
                            reg = gcnt_regs[(P % 3) * 2 * B + ci]
                            col = P * 2 * B + ci
                            nc.gpsimd.reg_load(
                                reg, ncnt_sb[0:1, col:col + 1])
                            nc.gpsimd.dma_gather(
                                g[:, roff:roff + rl, :],
                                tab[b * cfg.block_rows:(b + 1) * cfg.block_rows, :],
                                idx_sb[:, call["i16off"]:call["i16off"] + rl * 8],
                                rl * 128, reg, rowN, elem_step=rowN,
                                queue_num=ci % nc.num_swdge_queues)
                        choff = p["choff"]
                        # one-hot, chunk-dim last (2x is_equal), per run
                        S = psalt.tile([128, 128, CBP], BF16, tag="S")
                        for b, t, rl, roff in p["runs"]:
                            nc.vector.tensor_tensor(
                                out=S[:, :, roff:roff + rl],
                                in0=iotar_sb[:, :, :rl],
                                in1=dlp_sb[:, choff + roff:choff + roff + rl]
                                    .unsqueeze(1).broadcast_to([128, 128, rl]),
                                op=mybir.AluOpType.is_equal)
                        # S2 (transposed one-hot) + psE per run
                        psE = ppsE.tile([128, CBP, 2, HN], F32, tag="psE")
                        for b, t, rl, roff in p["runs"]:
                            psS2 = pps2.tile([128, RL, 128], BF16, tag="psS2")
                            for jj in range(rl):
                                nc.tensor.transpose(psS2[:, jj, :],
                                                    S[:, :, roff + jj],
                                                    ident_sb[:])
                            S2 = ps2p.tile([128, RL, 128], BF16, tag="S2")
                            nc.scalar.copy(S2[:, :rl, :], psS2[:, :rl, :])
                            for jj in range(rl):
                                nc.tensor.matmul(
                                    out=psE[:, roff + jj, :, :],
                                    lhsT=S2[:, jj, :],
                                    rhs=a_d_sb[:, t, :, :].rearrange(
                                        "p a b -> p (a b)"),
                                    start=True, stop=True)
                        # es / ee (pair-wide)
                        es = pep.tile([128, CBP, HN], F32, tag="es")
                        nc.vector.tensor_tensor(
                            out=es[:, :cbP, :], in0=psE[:, :cbP, 0, :],
                            in1=g[:, :cbP, as_off:as_off + HN],
                            op=mybir.AluOpType.add)
                        nc.vector.tensor_tensor(
                            out=es[:, :cbP, :], in0=psE[:, :cbP, 1, :],
                            in1=es[:, :cbP, :], op=mybir.AluOpType.add)
                        nc.vector.scalar_tensor_tensor(
                            out=es[:, :cbP, :], in0=es[:, :cbP, :],
                            scalar=cfg.neg_slope, in1=es[:, :cbP, :],
                            op0=mybir.AluOpType.mult, op1=mybir.AluOpType.max)
                        ee = pep.tile([128, CBP, HN], BF16, tag="ee")
                        nc.scalar.activation(ee[:, :cbP, :], es[:, :cbP, :],
                                             mybir.ActivationFunctionType.Exp,
                                             bias=mneg_bias[:], scale=1.0)
                        # msgee (pair-wide)
                        msgee = pmsg.tile([128, CBP, msg_w], BF16, tag="msgee")
                        if HN > 1:   # layer 1: channel-major 2x mults
                            nc.vector.tensor_tensor(
                                out=msgee[:, :cbP, 0:NBF].rearrange(
                                    "p c (a h) -> p c a h", h=HN),
                                in0=g[:, :cbP, 0:NBF].rearrange(
                                    "p c (a h) -> p c a h", h=HN),
                                in1=ee[:, :cbP, :].unsqueeze(2)
                                    .broadcast_to([128, cbP, NBF // HN, HN]),
                                op=mybir.AluOpType.mult)
                            gfp = pep.tile([128, CBP, NF8], BF16, tag="gfp")
                            nc.vector.tensor_copy(
                                gfp[:, :cbP, :],
                                g[:, :cbP, NBF + H:rowN].bitcast(FP8))
                            nc.vector.tensor_tensor(
                                out=msgee[:, :cbP, NBF:DN].rearrange(
                                    "p c (a h) -> p c a h", h=HN),
                                in0=gfp[:, :cbP, :].rearrange(
                                    "p c (a h) -> p c a h", h=HN),
                                in1=ee[:, :cbP, :].unsqueeze(2)
                                    .broadcast_to([128, cbP, NF8 // HN, HN]),
                                op=mybir.AluOpType.mult)
                        else:        # layer 2: single head
                            nc.vector.tensor_tensor(
                                out=msgee[:, :cbP, 0:DN],
                                in0=g[:, :cbP, 0:DN],
                                in1=ee[:, :cbP, :].broadcast_to([128, cbP, DN]),
                                op=mybir.AluOpType.mult)
                        nc.vector.tensor_copy(msgee[:, :cbP, DN:msg_w],
                                              ee[:, :cbP, :])
                        # aggregate + post per half
                        for half in (0, 1):
                            t = 2 * P + half
                            L = p["tile_chunks"][t]
                            psG = ppsG.tile([128, msg_w], F32, tag="psG")
                            for i, jg in enumerate(L):
                                nc.tensor.matmul(out=psG,
                                                 lhsT=S[:, :, jg],
                                                 rhs=msgee[:, jg, :],
                                                 start=(i == 0),
                                                 stop=(i == len(L) - 1))
                            post_tile(t, psG, ppost, ppsX)

            # ---- layer-1 post-tile ----
            def post1(t, psG, pep, pps):
                den = pep.tile([128, H], F32, tag="den")
                nc.vector.tensor_scalar_add(den, psG[:, D1:D1 + H], 1e-30)
                rden = pep.tile([128, H], F32, tag="rden")
                nc.vector.reciprocal(rden, den[:])
                z = pep.tile([128, D1], F32, tag="z")
                nc.vector.tensor_tensor(
                    out=z[:, 0:NBF].rearrange("p (a h) -> p a h", h=H),
                    in0=psG[:, 0:NBF].rearrange("p (a h) -> p a h", h=H),
                    in1=rden[:].unsqueeze(1).broadcast_to([128, NBF // H, H]),
                    op=mybir.AluOpType.mult)
                nc.vector.tensor_tensor(
                    out=z[:, NBF:D1].rearrange("p (a h) -> p a h", h=H),
                    in0=psG[:, NBF:D1].rearrange("p (a h) -> p a h", h=H),
                    in1=rden[:].unsqueeze(1).broadcast_to([128, NF8 // H, H]),
                    op=mybir.AluOpType.mult)
                nc.vector.tensor_tensor(out=z, in0=z[:], in1=b1_sb[:],
                                        op=mybir.AluOpType.add)
                zm = pep.tile([128, D1], F32, tag="zm")
                nc.vector.tensor_scalar_min(zm, z[:], 0.0)
                em = pep.tile([128, D1], F32, tag="em")
                nc.scalar.activation(em, zm[:], mybir.ActivationFunctionType.Exp,
                                     bias=zero_b[:])
                zp = pep.tile([128, D1], F32, tag="zp")
                nc.vector.tensor_scalar_max(zp, z[:], 0.0)
                elu_bf = pep.tile([128, D1], BF16, tag="elubf")
                nc.vector.scalar_tensor_tensor(
                    out=elu_bf, in0=em[:], scalar=-1.0, in1=zp[:],
                    op0=mybir.AluOpType.add, op1=mybir.AluOpType.add)
                eluT = pep.tile([ck, nk, 128], BF16, tag="eluT")
                for kk in range(nk):
                    psT = pps.tile([ck, 128], BF16, tag="psmisc")
                    nc.tensor.transpose(psT, elu_bf[:, kk * ck:(kk + 1) * ck],
                                        ident_sb[:])
                    nc.scalar.copy(eluT[:, kk, :], psT[:])
                psH2 = pps.tile([128, C + 2], F32, tag="psmisc")
                for kk in range(nk):
                    nc.tensor.matmul(out=psH2, lhsT=eluT[:, kk, :],
                                     rhs=W2e_sb[:, kk, :],
                                     start=(kk == 0), stop=(kk == nk - 1))
                row = pep.tile([128, row2], BF16, tag="rowC")
                if t < 2:
                    nc.vector.memset(row[:, C + 1:], 0.0)
                nc.scalar.copy(row[:, 0:C + 1], psH2[:, 0:C + 1])
                nc.vector.tensor_copy(a_d2_sb[:, t, 0, :], psH2[:, C + 1:C + 2])
                nc.vector.tensor_tensor(out=a_d2_sb[:, t, 1, :],
                                        in0=psH2[:, C + 1:C + 2],
                                        in1=a_d2_sb[:, t, 0, :],
                                        op=mybir.AluOpType.subtract)
                nc.sync.dma_start(out=tab2_sh[t * 128:(t + 1) * 128, :],
                                  in_=row[:])

            edge_phase(tab1, row1, a_d1_sb, H, mneg1_b, D1 + H, NBF, post1)

            if cfg.n_cores == 1:
                nc.sync.dma_start(out=tab2[:], in_=tab2_sh[:])
            elif cfg.sim1:
                for k in range(cfg.n_cores):
                    nc.sync.dma_start(out=tab2[k * npc:(k + 1) * npc, :],
                                      in_=tab2_sh[:])
            else:
                nc.gpsimd.collective_compute(
                    "AllGather", mybir.AluOpType.bypass, groups,
                    ins=[tab2_sh[:]], outs=[tab2[:]])

            # ---- layer-2 post-tile (stores zs; Ln batched later) ----
            def post2(t, psG, pep, pps):
                den = pep.tile([128, 1], F32, tag="den2")
                nc.vector.tensor_scalar_add(den, psG[:, C:C + 1], 1e-30)
                rden = pep.tile([128, 1], F32, tag="rden2")
                nc.vector.reciprocal(rden, den[:])
                z = pep.tile([128, C], F32, tag="z2")
                nc.vector.tensor_scalar(out=z, in0=psG[:, 0:C], scalar1=rden[:],
                                        scalar2=None, op0=mybir.AluOpType.mult)
                nc.vector.tensor_tensor(out=z, in0=z[:], in1=b2_sb[:],
                                        op=mybir.AluOpType.add)
                zm = pep.tile([128, C], F32, tag="zm2")
                nc.vector.tensor_scalar_min(zm, z[:], 0.0)
                em = pep.tile([128, C], F32, tag="em2")
                nc.scalar.activation(em, zm[:], mybir.ActivationFunctionType.Exp,
                                     bias=zero_b[:])
                zp = pep.tile([128, C], F32, tag="zp2")
                nc.vector.tensor_scalar_max(zp, z[:], 0.0)
                h3 = pep.tile([128, C], BF16, tag="h3")
                nc.vector.scalar_tensor_tensor(
                    out=h3, in0=em[:], scalar=-1.0, in1=zp[:],
                    op0=mybir.AluOpType.add, op1=mybir.AluOpType.add)
                psT = pps.tile([C, 128], BF16, tag="psmisc2")
                nc.tensor.transpose(psT, h3[:], ident_sb[:])
                h3T = pep.tile([C, 128], BF16, tag="h3T")
                nc.scalar.copy(h3T, psT[:])
                psL = pps.tile([128, cfg.classes], F32, tag="psmisc2")
                nc.tensor.matmul(out=psL, lhsT=h3T[:], rhs=Wout_sb[:],
                                 start=True, stop=True)
                z3 = pep.tile([128, cfg.classes], F32, tag="z3")
                nc.vector.tensor_tensor(out=z3, in0=psL[:], in1=bout_sb[:],
                                        op=mybir.AluOpType.add)
                mx = pep.tile([128, 1], F32, tag="mx")
                nc.vector.tensor_reduce(mx, z3[:], axis=mybir.AxisListType.X,
                                        op=mybir.AluOpType.max)
                zs = pep.tile([128, cfg.classes], F32, tag="zsf")
                nc.vector.tensor_scalar(out=zs, in0=z3[:], scalar1=mx[:],
                                        scalar2=None,
                                        op0=mybir.AluOpType.subtract)
                es3 = pep.tile([128, cfg.classes], F32, tag="es3")
                nc.scalar.activation(es3, zs[:],
                                     mybir.ActivationFunctionType.Exp,
                                     bias=zero_b[:],
                                     accum_out=ssum_sb[:, t:t + 1])
                nc.vector.tensor_copy(zs_sb[:, t, :], zs[:])

            edge_phase(tab2, row2, a_d2_sb, 1, mneg2_b, C + 1, C, post2)

            # ---- epilogue: batched Ln + output ----
            with tc.tile_pool(name="ep", bufs=2) as pe2:
                lg = pe2.tile([128, T], F32, tag="lg")
                nc.scalar.activation(lg, ssum_sb[:],
                                     mybir.ActivationFunctionType.Ln,
                                     bias=zero_b[:])
                for t in range(T):
                    oo = pe2.tile([128, cfg.classes], F32, tag="oo")
                    nc.vector.tensor_scalar(out=oo, in0=zs_sb[:, t, :],
                                            scalar1=lg[:, t:t + 1], scalar2=None,
                                            op0=mybir.AluOpType.subtract)
                    nc.sync.dma_start(out=out[t * 128:(t + 1) * 128, :], in_=oo[:])

    nc.compile()
    return nc


# ============================ public entry point ============================

_CACHE = {}


def kernel(**inputs):
    import numpy as np
    cfg = Cfg()
    hd = prep_host_data(cfg, inputs)
    key = ("gat2", cfg.n_pad, cfg.total_chunks, cfg.total_idx16,
           tuple(cfg.c_tb.reshape(-1).tolist()))
    nc = _CACHE.get(key)
    if nc is None:
        nc = build_program(cfg)
        _CACHE.clear()
        _CACHE[key] = nc
    from concourse.bass_utils import run_bass_kernel_spmd
    res = None
    last_err = None
    for _attempt in range(3):
        try:
            res = run_bass_kernel_spmd(nc, hd.per_core, list(range(cfg.n_cores)))
            break
        except Exception as e:
            last_err = e
    if res is None:
        raise last_err
    full = np.concatenate([res.results[i]["out"] for i in range(cfg.n_cores)],
                          axis=0)
    out = full[hd.perm[:cfg.n_nodes]]
    return np.ascontiguousarray(out.astype(np.float32))
